# revision 1
# baseline (speedup 1.0000x reference)
"""Trainium2 Bass kernel for CustomTriangleMultiplicationOutgoing.

Reference computation (B=1, N=384, D=C=128):
    z_norm = LN(z) * g + b                        # over D
    left   = (z_norm@Wa + ba) * sigmoid(z_norm@Wga + bga) * mask
    right  = (z_norm@Wb + bb) * sigmoid(z_norm@Wgb + bgb) * mask
    z_out[i,j,c] = sum_k left[i,k,c] * right[j,k,c]
    z_out  = LN(z_out) * g_out + b_out            # over C
    out    = (z_out@Wo + bo) * sigmoid(z_norm@Wgo + bgo)

Key identity: row-wise LN commutes with the projection,
    LN(z) @ (g .* W) = (z * rstd) @ ((I - 11^T/D)(g .* W))
so the host passes zs = (z * rstd)^T in bf16 and centered/affine-folded
weights; the device does plain matmuls with NO LN work in phase 1.
Phase 3's LN over C uses the same centering on Wo; its mean comes from a
vector-accumulated column sum during phase 2, its mean-square from
partial-K matmuls against a ones vector fused into each phase-2 chunk.

Sharding: 1D over the first N (i) axis, 48 rows per core.  Pass A
computes only the gated right projection so the 4 c-chunked AllGathers
(bf16) start early; pass B (left + out-gate) and phase 2 (einsum, k on
partitions) hide under the collectives.  Pass A/B element-wise ops are
batched over pairs of row tiles to amortize per-instruction overhead.
"""

import numpy as np
import ml_dtypes

import concourse.bass as bass
import concourse.mybir as mybir
import concourse.tile as tile
from concourse import bacc
from concourse.masks import make_identity
from concourse.bass_utils import run_bass_kernel_spmd

F32 = mybir.dt.float32
BF16 = mybir.dt.bfloat16
EPS = 1e-5

B = 1
N_FULL = 384
D = 128
C = 128
W = 8  # cores
P = 128


def bcast_part(ap, parts):
    """Broadcast a [1, ...] AP across `parts` partitions (partition step 0)."""
    return bass.AP(tensor=ap.tensor, offset=ap.offset, ap=[[0, parts]] + ap.ap[1:])


def pair_ap(ap0, ap1):
    """Fuse two same-shape/stride APs into one with a [delta, 2] middle dim."""
    assert ap0.ap == ap1.ap and ap0.tensor is ap1.tensor
    delta = ap1.offset - ap0.offset
    return bass.AP(
        tensor=ap0.tensor, offset=ap0.offset,
        ap=[ap0.ap[0]] + [[delta, 2]] + ap0.ap[1:],
    )


def build_nc(n=N_FULL, with_bias=False, with_mask=False, nq=4):
    """Build the SPMD Bass program (same program on all 8 cores)."""
    assert n % P == 0 and n % W == 0
    SH = n // W          # rows of i per core
    KC = n // P          # 128-wide chunks of k
    NT = SH * n // P     # 128-row tiles per core (= SH*KC)
    CQ = C // nq         # c per AllGather chunk
    CQH = CQ // 2        # c per phase-2 half-load
    QP = CQ              # partitions per chunk in zt_all (c-sharded rows)

    nc = bacc.Bacc(None, num_devices=W)

    zs = nc.declare_dram_parameter("zs", [P, NT * P], BF16, isOutput=False)
    wbg = nc.declare_dram_parameter("wbg", [D, 2 * C], BF16, isOutput=False)
    wago = nc.declare_dram_parameter("wago", [D, 2 * C + D], BF16, isOutput=False)
    wo = nc.declare_dram_parameter("wo", [C, D], BF16, isOutput=False)
    if with_bias:
        bbg_p = nc.declare_dram_parameter("bbg", [1, 2 * C], F32, isOutput=False)
        bago_p = nc.declare_dram_parameter("bago", [1, 2 * C + D], F32, isOutput=False)
        bo_p = nc.declare_dram_parameter("bo", [1, D], F32, isOutput=False)
    if with_mask:
        mask_sh = nc.declare_dram_parameter("mask_sh", [P, NT], F32, isOutput=False)
    out_sh = nc.declare_dram_parameter("out_sh", [P, NT, D], F32, isOutput=True)

    # internal DRAM
    right_q = [nc.dram_tensor(f"right_{q}", [P, KC, CQ, SH], BF16) for q in range(nq)]
    gath_q = [
        nc.dram_tensor(f"gath_{q}", [W, P, KC, CQ, SH], BF16, addr_space="Shared")
        for q in range(nq)
    ]
    zout = nc.dram_tensor("zout", [C, SH * n], BF16)  # c-major einsum result

    with tile.TileContext(nc) as tc:
        with tc.tile_pool(name="singles", bufs=1) as singles:
            wbg_sb = singles.tile([D, 2 * C], BF16)
            nc.sync.dma_start(wbg_sb, wbg[:])
            wago_sb = singles.tile([D, 2 * C + D], BF16)
            nc.sync.dma_start(wago_sb, wago[:])
            wo_sb = singles.tile([C, D], BF16)
            nc.sync.dma_start(wo_sb, wo[:])
            ones_bf = singles.tile([P, 1], BF16)
            nc.vector.memset(ones_bf, 1.0)
            eps_sb = singles.tile([P, 1], F32)
            nc.vector.memset(eps_sb, EPS)
            ident = singles.tile([P, P], F32)
            make_identity(nc, ident)
            if with_bias:
                bbg_sb = singles.tile([P, 2 * C], F32)
                nc.sync.dma_start(bbg_sb, bcast_part(bbg_p[:], P))
                bago_sb = singles.tile([P, 2 * C + D], F32)
                nc.sync.dma_start(bago_sb, bcast_part(bago_p[:], P))
                bo_sb = singles.tile([P, D], F32)
                nc.sync.dma_start(bo_sb, bcast_part(bo_p[:], P))
            if with_mask:
                mask_sb = singles.tile([P, NT], F32)
                nc.sync.dma_start(mask_sb, mask_sh[:])

            # persistent stores
            gg_sb = singles.tile([P, NT, D], BF16)      # out-gate per row tile
            zt_all = singles.tile([C, NT * P], BF16)    # z_out, c on partitions
            S_acc = singles.tile([SH, n], F32)          # sum_c z_out
            nc.vector.memset(S_acc, 0.0)

            lpool = tc.alloc_tile_pool(name="lpool", bufs=1)
            L_sb = lpool.tile([P, KC, SH, C], BF16)     # left: [k, kc, i_loc, c]

            p1pool = tc.alloc_tile_pool(name="p1", bufs=1)
            zs_sb = p1pool.tile([P, NT * P], BF16)
            for ch in range(8):
                w8 = NT * P // 8
                nc.sync.dma_start(
                    zs_sb[:, ch * w8 : (ch + 1) * w8],
                    zs[:, ch * w8 : (ch + 1) * w8],
                )
            R_stage = p1pool.tile([P, KC, C, SH], BF16)  # right: [k, kc, c, j_loc]

            NPAIR = NT // 2

            # ---------------- pass A: right projection ----------------
            with (
                tc.tile_pool(name="pA_tmp", bufs=4) as tmpsA,
                tc.tile_pool(name="pA_psum", bufs=4, space="PSUM") as psumA,
            ):
                for pi in range(NPAIR):
                    t0, t1 = 2 * pi, 2 * pi + 1
                    ps = psumA.tile([P, 2, 2 * C], F32, tag="psA")
                    for j, t in enumerate((t0, t1)):
                        nc.tensor.matmul(
                            ps[:, j, :], lhsT=zs_sb[:, t * P : (t + 1) * P],
                            rhs=wbg_sb, start=True, stop=True,
                        )
                    if with_bias:
                        for j in range(2):
                            nc.vector.tensor_tensor(
                                ps[:, j, :], ps[:, j, :], bbg_sb,
                                mybir.AluOpType.add,
                            )
                    sgb = tmpsA.tile([P, 2, C], BF16, tag="sgb")
                    nc.scalar.activation(
                        sgb, ps[:, :, C : 2 * C],
                        mybir.ActivationFunctionType.Sigmoid,
                    )
                    bcp = tmpsA.tile([P, 2, C], BF16, tag="bcp")
                    nc.scalar.copy(bcp, ps[:, :, 0:C])
                    if with_mask:
                        for j, t in enumerate((t0, t1)):
                            nc.gpsimd.tensor_scalar_mul(
                                sgb[:, j, :], sgb[:, j, :], mask_sb[:, t : t + 1]
                            )
                    rout = pair_ap(
                        R_stage[:, t0 % KC, :, t0 // KC],
                        R_stage[:, t1 % KC, :, t1 // KC],
                    )
                    nc.vector.tensor_tensor(
                        rout, bcp, sgb, mybir.AluOpType.mult
                    )
                for q in range(nq):
                    nc.sync.dma_start(
                        right_q[q][:], R_stage[:, :, q * CQ : (q + 1) * CQ, :]
                    )

            # ---------------- AllGather right (c-chunked) ----------------
            for q in range(nq):
                nc.gpsimd.collective_compute(
                    "AllGather",
                    mybir.AluOpType.bypass,
                    replica_groups=[list(range(W))],
                    ins=[right_q[q][:]],
                    outs=[gath_q[q][:]],
                )

            # ---------------- pass B: left + out-gate ----------------
            with (
                tc.tile_pool(name="pB_tmp", bufs=4) as tmpsB,
                tc.tile_pool(name="pB_psum", bufs=4, space="PSUM") as psumB,
                tc.tile_pool(name="pG_psum", bufs=4, space="PSUM") as psumG,
            ):
                for pi in range(NPAIR):
                    t0, t1 = 2 * pi, 2 * pi + 1
                    ps = psumB.tile([P, 2, 2 * C], F32, tag="psB")
                    pg = psumG.tile([P, 2, D], F32, tag="psG")
                    for j, t in enumerate((t0, t1)):
                        nc.tensor.matmul(
                            ps[:, j, :], lhsT=zs_sb[:, t * P : (t + 1) * P],
                            rhs=wago_sb[:, 0 : 2 * C], start=True, stop=True,
                        )
                        nc.tensor.matmul(
                            pg[:, j, :], lhsT=zs_sb[:, t * P : (t + 1) * P],
                            rhs=wago_sb[:, 2 * C :], start=True, stop=True,
                        )
                    if with_bias:
                        for j in range(2):
                            nc.vector.tensor_tensor(
                                ps[:, j, :], ps[:, j, :], bago_sb[:, 0 : 2 * C],
                                mybir.AluOpType.add,
                            )
                            nc.vector.tensor_tensor(
                                pg[:, j, :], pg[:, j, :], bago_sb[:, 2 * C :],
                                mybir.AluOpType.add,
                            )
                    sga = tmpsB.tile([P, 2, C], F32, tag="sga")
                    nc.scalar.activation(
                        sga, ps[:, :, C : 2 * C],
                        mybir.ActivationFunctionType.Sigmoid,
                    )
                    nc.scalar.activation(
                        gg_sb[:, t0 : t0 + 2, :], pg,
                        mybir.ActivationFunctionType.Sigmoid,
                    )
                    if with_mask:
                        for j, t in enumerate((t0, t1)):
                            nc.gpsimd.tensor_scalar_mul(
                                sga[:, j, :], sga[:, j, :], mask_sb[:, t : t + 1]
                            )
                    lout = pair_ap(
                        L_sb[:, t0 % KC, t0 // KC, :],
                        L_sb[:, t1 % KC, t1 // KC, :],
                    )
                    nc.vector.tensor_tensor(
                        lout, ps[:, :, 0:C], sga, mybir.AluOpType.mult
                    )

            p1pool.release()  # zs, R_stage freed

            # ---------------- phase 2: einsum + fused z_out stats ----------------
            sqpool = tc.alloc_tile_pool(name="sq_psum", bufs=1, space="PSUM")
            # one column block per chunk-pair, self-contained matmul chains
            sq_ps = sqpool.tile([P, 2, NT], F32)  # sum_c z_out^2 per row tile
            with (
                tc.tile_pool(name="p2_r", bufs=2) as rpool,
                tc.tile_pool(name="p2_st", bufs=3) as stpool,
                tc.tile_pool(name="p2_sq", bufs=4) as sqtmp,
                tc.tile_pool(name="p2_psum", bufs=6, space="PSUM") as psum2,
            ):
                for q in range(nq):
                    for h in range(2):
                        Rh = rpool.tile([P, KC, W, CQH, SH], BF16, tag="rh")
                        for m in range(W):
                            nc.sync.dma_start(
                                Rh[:, :, m],
                                gath_q[q][m, :, :, h * CQH : (h + 1) * CQH, :],
                            )
                        for c4 in range(CQH // 4):
                            stb = stpool.tile([SH, 4, n], BF16, tag="stb")
                            for c_ in range(4):
                                cl = c4 * 4 + c_
                                c_glob = q * CQ + h * CQH + cl
                                ps = psum2.tile([SH, n], F32, tag="ps")
                                for kc in range(KC):
                                    nc.tensor.matmul(
                                        ps,
                                        lhsT=L_sb[:, kc, :, c_glob],
                                        rhs=Rh[:, kc, :, cl, :],
                                        start=(kc == 0),
                                        stop=(kc == KC - 1),
                                    )
                                if c_ % 2 == 0:
                                    nc.vector.tensor_copy(stb[:, c_, :], ps)
                                else:
                                    nc.scalar.copy(stb[:, c_, :], ps)
                                nc.vector.tensor_tensor(
                                    S_acc, S_acc, ps, mybir.AluOpType.add
                                )
                            c0 = q * CQ + h * CQH + c4 * 4
                            nc.sync.dma_start(
                                zout[c0 : c0 + 4].rearrange(
                                    "c (i j) -> i c j", i=SH
                                ),
                                stb,
                            )
                    # z_out c-rows for this chunk are final: fetch to SBUF
                    nc.sync.dma_start(
                        zt_all[q * QP : (q + 1) * QP, :],
                        zout[q * QP : (q + 1) * QP, :],
                    )
                    # fused partial mean-square over pairs of chunks
                    # (matmul base partition must be 0/32/64)
                    if q % 2 == 1:
                        lo = (q - 1) * QP
                        for t in range(NT):
                            zq = zt_all[lo : lo + 2 * QP, t * P : (t + 1) * P]
                            sqv = sqtmp.tile([P, P], BF16, tag="sqv")
                            sqs = sqv[lo : lo + 2 * QP, :]
                            nc.vector.tensor_tensor(
                                sqs, zq, zq, mybir.AluOpType.mult
                            )
                            nc.tensor.matmul(
                                sq_ps[:, q // 2, t : t + 1], lhsT=sqs,
                                rhs=ones_bf[lo : lo + 2 * QP, :],
                                start=True, stop=True,
                            )

            SQm = singles.tile([P, NT], F32)   # sum_c z_out^2, tile-major
            nc.vector.tensor_copy(SQm, sq_ps[:, 0, :])
            nc.vector.tensor_tensor(
                SQm, SQm, sq_ps[:, 1, :], mybir.AluOpType.add
            )
            sqpool.release()
            lpool.release()  # L_sb freed

            # ---------------- phase 3: LN(z_out) @ Wo * gate ----------------
            with (
                tc.tile_pool(name="p3_tmp", bufs=4) as t3,
                tc.tile_pool(name="p3_big", bufs=1) as big3,
                tc.tile_pool(name="p3_psum", bufs=4, space="PSUM") as psum3,
                tc.tile_pool(name="p3_tps", bufs=3, space="PSUM") as tpsum,
            ):
                # transpose S_acc [SH, n] -> Smat [P, NT] (tile-major stats)
                Smat = big3.tile([P, NT], F32)
                for jc in range(KC):
                    tp = tpsum.tile([P, SH], F32, tag="tp")
                    nc.tensor.transpose(
                        tp, S_acc[:, jc * P : (jc + 1) * P], ident[0:SH, 0:SH]
                    )
                    nc.vector.tensor_copy(
                        Smat[:].rearrange("p (i k) -> p k i", k=KC)[:, jc, :], tp
                    )
                mean = big3.tile([P, NT], F32)
                nc.vector.tensor_scalar_mul(mean, Smat, 1.0 / C)
                msq = big3.tile([P, NT], F32)
                nc.vector.tensor_scalar_mul(msq, SQm, 1.0 / C)
                var = big3.tile([P, NT], F32)
                nc.vector.tensor_tensor(var, mean, mean, mybir.AluOpType.mult)
                nc.vector.tensor_tensor(var, msq, var, mybir.AluOpType.subtract)
                rstd = big3.tile([P, NT], F32)
                nc.scalar.activation(
                    rstd, var, mybir.ActivationFunctionType.Sqrt, bias=eps_sb
                )
                nc.vector.reciprocal(rstd, rstd)

                ot_sb = big3.tile([P, NT, D], F32)
                for t in range(NT):
                    pr = psum3.tile([P, D], F32, tag="pr")
                    nc.tensor.matmul(
                        pr, lhsT=zt_all[:, t * P : (t + 1) * P], rhs=wo_sb,
                        start=True, stop=True,
                    )
                    if with_bias:
                        po = t3.tile([P, D], F32, tag="po")
                        nc.vector.tensor_scalar_mul(po, pr, rstd[:, t : t + 1])
                        nc.vector.tensor_tensor(po, po, bo_sb, mybir.AluOpType.add)
                        nc.vector.tensor_tensor(
                            ot_sb[:, t, :], po, gg_sb[:, t, :], mybir.AluOpType.mult
                        )
                    else:
                        nc.vector.scalar_tensor_tensor(
                            ot_sb[:, t, :], pr, rstd[:, t : t + 1],
                            gg_sb[:, t, :],
                            mybir.AluOpType.mult, mybir.AluOpType.mult,
                        )
                for ch in range(4):
                    t0 = NT // 4 * ch
                    t1 = NT // 4 * (ch + 1)
                    nc.sync.dma_start(
                        out_sh[:, t0:t1, :], ot_sb[:, t0:t1, :]
                    )

    nc.compile()
    return nc


_CACHE = {}


def _get_nc(n, with_bias, with_mask):
    key = (n, with_bias, with_mask)
    if key not in _CACHE:
        _CACHE[key] = build_nc(n=n, with_bias=with_bias, with_mask=with_mask)
    return _CACHE[key]


def prepare_host(z, mask, norm_g, norm_b, norm_out_g, norm_out_b,
                 Wa, ba, Wb, bb, Wga, bga, Wgb, bgb, Wo, bo, Wgo, bgo, n=N_FULL):
    """Fold norm affines + centering into weights; pre-normalize z rows."""
    f = np.asarray
    z = f(z, dtype=np.float32)
    mask = f(mask, dtype=np.float32)
    g = f(norm_g, np.float32)
    b = f(norm_b, np.float32)
    go = f(norm_out_g, np.float32)
    bo_n = f(norm_out_b, np.float32)

    # LN(z) @ W_aff + bias = (z*rstd) @ Wcen + (b @ W + bias),
    # Wcen = (I - J/D)(g .* W)
    def fold(Wm, bias):
        Wm = f(Wm, np.float32)
        Wg = g[:, None] * Wm
        Wcen = Wg - np.mean(Wg, axis=0, keepdims=True)
        return Wcen, f(bias, np.float32) + b @ Wm

    Wa_, ba_ = fold(Wa, ba)
    Wga_, bga_ = fold(Wga, bga)
    Wb_, bb_ = fold(Wb, bb)
    Wgb_, bgb_ = fold(Wgb, bgb)
    Wgo_, bgo_ = fold(Wgo, bgo)
    Wo32 = f(Wo, np.float32)
    Wog = go[:, None] * Wo32
    Wo_ = Wog - np.mean(Wog, axis=0, keepdims=True)
    bo_ = f(bo, np.float32) + bo_n @ Wo32

    bf = ml_dtypes.bfloat16
    wbg_h = np.concatenate([Wb_, Wgb_], axis=1).astype(bf)
    wago_h = np.concatenate([Wa_, Wga_, Wgo_], axis=1).astype(bf)
    wo_h = Wo_.astype(bf)
    bbg_h = np.concatenate([bb_, bgb_])[None, :].astype(np.float32)
    bago_h = np.concatenate([ba_, bga_, bgo_])[None, :].astype(np.float32)

    with_bias = bool(np.any(bbg_h) or np.any(bago_h) or np.any(bo_))
    with_mask = not bool(np.all(mask == 1.0))

    # host-side LN stats: rstd per row of z, folded into z itself
    zf = z[0].reshape(n * n, D)
    m = zf.mean(axis=1, keepdims=True)
    v = ((zf - m) ** 2).mean(axis=1, keepdims=True)
    r = 1.0 / np.sqrt(v + EPS)
    zsf = (zf * r).astype(np.float32)

    SH = n // W
    NT = SH * n // P
    in_maps = []
    for mi in range(W):
        rows = zsf[SH * n * mi : SH * n * (mi + 1)]  # [SH*n, D]
        zs_h = np.ascontiguousarray(rows.T).astype(bf)  # [D, SH*n]
        im = {
            "zs": zs_h,
            "wbg": wbg_h,
            "wago": wago_h,
            "wo": wo_h,
        }
        if with_bias:
            im["bbg"] = bbg_h
            im["bago"] = bago_h
            im["bo"] = bo_[None, :].astype(np.float32)
        if with_mask:
            msk = mask[0].reshape(n * n)[SH * n * mi : SH * n * (mi + 1)]
            im["mask_sh"] = np.ascontiguousarray(
                msk.reshape(NT, P).T
            ).astype(np.float32)
        in_maps.append(im)
    return in_maps, with_bias, with_mask


def unshard(results, n=N_FULL):
    """results: list of per-core out_sh arrays [P, NT, D] -> [1, n, n, D]."""
    SH = n // W
    NT = SH * n // P
    parts = []
    for mi in range(W):
        o = results[mi].reshape(P, NT, D)
        parts.append(o.transpose(1, 0, 2).reshape(SH, n, D))
    return np.concatenate(parts, axis=0)[None]


def kernel(**inputs):
    n = inputs["z"].shape[1]
    in_maps, with_bias, with_mask = prepare_host(**inputs, n=n)
    nc = _get_nc(n, with_bias, with_mask)
    res = run_bass_kernel_spmd(nc, in_maps, list(range(W)))
    out = unshard([res.results[m]["out_sh"] for m in range(W)], n=n)
    return out.astype(np.float32)



# revision 12
# speedup vs baseline: 1.1039x; 1.1039x over previous
"""Trainium2 Bass kernel for CustomTriangleMultiplicationOutgoing.

Reference computation (B=1, N=384, D=C=128):
    z_norm = LN(z) * g + b                        # over D
    left   = (z_norm@Wa + ba) * sigmoid(z_norm@Wga + bga) * mask
    right  = (z_norm@Wb + bb) * sigmoid(z_norm@Wgb + bgb) * mask
    z_out[i,j,c] = sum_k left[i,k,c] * right[j,k,c]
    z_out  = LN(z_out) * g_out + b_out            # over C
    out    = (z_out@Wo + bo) * sigmoid(z_norm@Wgo + bgo)

Key identity: row-wise LN commutes with the projection,
    LN(z) @ (g .* W) = (z * rstd) @ ((I - 11^T/D)(g .* W))
so the host passes zs = (z * rstd)^T in bf16 and centered/affine-folded
weights; the device does plain matmuls with NO LN work in phase 1.

Sharding: phase 1 is row-sharded (48 rows of i per core); phase 2 (the
einsum) is CHANNEL-sharded: an AllToAll exchanges left+right so each core
holds all (i,k) for 16 channels, giving full 128-row PE utilization in
the einsum matmuls (lhsT [128k x 128i] @ rhs [128k x 384j]).  A second
AllToAll brings z_out back to row-sharding with c on partitions, where
the output LN stats (sum / sum-sq over c) come from 1-column matmuls
against a ones vector, and the final projection is plain zt^T @ Wo.
The forward AllToAll is chunked by k (3 chunks) so it overlaps phase-1
compute; the backward one by c-halves so it overlaps phase 2.
"""

import numpy as np
import ml_dtypes

import concourse.bass as bass
import concourse.mybir as mybir
import concourse.tile as tile
from concourse import bacc
from concourse.bass_utils import run_bass_kernel_spmd

F32 = mybir.dt.float32
BF16 = mybir.dt.bfloat16
EPS = 1e-5

B = 1
N_FULL = 384
D = 128
C = 128
W = 8  # cores
P = 128


def bcast_part(ap, parts):
    """Broadcast a [1, ...] AP across `parts` partitions (partition step 0)."""
    return bass.AP(tensor=ap.tensor, offset=ap.offset, ap=[[0, parts]] + ap.ap[1:])


def build_nc(n=N_FULL, with_bias=False, with_mask=False):
    """Build the SPMD Bass program (same program on all 8 cores)."""
    assert n % P == 0 and n % W == 0
    SH = n // W          # rows of i per core (48)
    KC = n // P          # 128-wide chunks of k (3)
    NT = SH * KC         # 128-row tiles per core (144); tile t=(r,kc): t=r*KC+kc
    CL = C // W          # local channels per core in phase 2 (16)
    CH = CL // 2         # channels per backward-A2A half (8)
    NPAIR = SH // 2      # pairs of rows per kc sweep

    nc = bacc.Bacc(None, num_devices=W)

    zs = nc.declare_dram_parameter("zs", [P, NT * P], BF16, isOutput=False)
    wva = nc.declare_dram_parameter("wva", [D, 2 * C], BF16, isOutput=False)
    wg3 = nc.declare_dram_parameter("wg3", [D, 3 * C], BF16, isOutput=False)
    wo = nc.declare_dram_parameter("wo", [C, D], BF16, isOutput=False)
    if with_bias:
        bva_p = nc.declare_dram_parameter("bva", [1, 2 * C], F32, isOutput=False)
        bg3_p = nc.declare_dram_parameter("bg3", [1, 3 * C], F32, isOutput=False)
        bo_p = nc.declare_dram_parameter("bo", [1, D], F32, isOutput=False)
    if with_mask:
        mask_sh = nc.declare_dram_parameter("mask_sh", [P, NT], F32, isOutput=False)
    out_sh = nc.declare_dram_parameter("out_sh", [P, NT, D], F32, isOutput=True)

    # internal DRAM for the collectives
    # forward: one chunk per kc; block to dest g = [side, k, c_loc, i_loc]
    lra2a = [
        nc.dram_tensor(f"lra2a_{kc}", [W, 2, P, CL, SH], BF16) for kc in range(KC)
    ]
    ga2a = [
        nc.dram_tensor(f"ga2a_{kc}", [W, 2, P, CL, SH], BF16) for kc in range(KC)
    ]
    # backward: one chunk per c-half; block to dest g = [c_loc, i_loc, j]
    zoa2a = [nc.dram_tensor(f"zoa2a_{h}", [W, CH, SH, n], BF16) for h in range(2)]
    gza2a = [nc.dram_tensor(f"gza2a_{h}", [W, CH, SH, n], BF16) for h in range(2)]

    with tile.TileContext(nc) as tc:
        with tc.tile_pool(name="singles", bufs=1) as singles:
            wva_sb = singles.tile([D, 2 * C], BF16)
            nc.sync.dma_start(wva_sb, wva[:])
            wg3_sb = singles.tile([D, 3 * C], BF16)
            nc.sync.dma_start(wg3_sb, wg3[:])
            wo_sb = singles.tile([C, D], BF16)
            nc.sync.dma_start(wo_sb, wo[:])
            ones_bf = singles.tile([P, 1], BF16)
            nc.vector.memset(ones_bf, 1.0)
            eps_sb = singles.tile([P, 1], F32)
            nc.vector.memset(eps_sb, EPS)
            if with_bias:
                bva_sb = singles.tile([P, 2 * C], F32)
                nc.sync.dma_start(bva_sb, bcast_part(bva_p[:], P))
                bg3_sb = singles.tile([P, 3 * C], F32)
                nc.sync.dma_start(bg3_sb, bcast_part(bg3_p[:], P))
                bo_sb = singles.tile([P, D], F32)
                nc.sync.dma_start(bo_sb, bcast_part(bo_p[:], P))
            if with_mask:
                mask_sb = singles.tile([P, NT], F32)
                nc.sync.dma_start(mask_sb, mask_sh[:])

            # persistent store: out-gate projection; even pairs hold
            # sigmoid(go) (ACT wrote it), odd pairs hold raw go (DVE copy)
            go_raw = singles.tile([P, NT, D], BF16)

            # ------------- phase 1: fused projections, kc-major -------------
            p1pool = tc.alloc_tile_pool(name="p1", bufs=1)
            zs_sb = p1pool.tile([P, NT * P], BF16)
            for ch in range(8):
                w8 = NT * P // 8
                nc.sync.dma_start(
                    zs_sb[:, ch * w8 : (ch + 1) * w8],
                    zs[:, ch * w8 : (ch + 1) * w8],
                )
            # left/right staging: [k, kc, side(R=0,L=1), c, r]
            lr_loc = p1pool.tile([P, KC, 2, C, SH], BF16)

            with (
                tc.tile_pool(name="p1_sg", bufs=3) as sgpool,
                tc.tile_pool(name="p1_pv", bufs=2, space="PSUM") as pvpool,
                tc.tile_pool(name="p1_pg1", bufs=2, space="PSUM") as pg1pool,
                tc.tile_pool(name="p1_pg2", bufs=2, space="PSUM") as pg2pool,
            ):
                for kc in range(KC):
                    for rp in range(NPAIR):
                        r0 = 2 * rp
                        ts = [(r0 + j) * KC + kc for j in range(2)]
                        cols = [(r0 + j) * n + kc * P for j in range(2)]
                        pv = pvpool.tile([P, 2, 2 * C], F32, tag="pv")
                        pg1 = pg1pool.tile([P, 2, 2 * C], F32, tag="pg1")
                        pg2 = pg2pool.tile([P, 2, C], F32, tag="pg2")
                        for j in range(2):
                            lhsT = zs_sb[:, cols[j] : cols[j] + P]
                            nc.tensor.matmul(
                                pv[:, j, :], lhsT=lhsT, rhs=wva_sb,
                                start=True, stop=True,
                            )
                            nc.tensor.matmul(
                                pg1[:, j, :], lhsT=lhsT, rhs=wg3_sb[:, 0 : 2 * C],
                                start=True, stop=True,
                            )
                            nc.tensor.matmul(
                                pg2[:, j, :], lhsT=lhsT, rhs=wg3_sb[:, 2 * C :],
                                start=True, stop=True,
                            )
                        if with_bias:
                            for j in range(2):
                                nc.vector.tensor_tensor(
                                    pv[:, j, :], pv[:, j, :], bva_sb,
                                    mybir.AluOpType.add,
                                )
                                nc.vector.tensor_tensor(
                                    pg1[:, j, :], pg1[:, j, :], bg3_sb[:, 0 : 2 * C],
                                    mybir.AluOpType.add,
                                )
                                nc.vector.tensor_tensor(
                                    pg2[:, j, :], pg2[:, j, :], bg3_sb[:, 2 * C :],
                                    mybir.AluOpType.add,
                                )
                        # sigmoid of (gb|ga) for both rows in one ACT call
                        sg1 = sgpool.tile([P, 2, 2 * C], BF16, tag="sg1")
                        nc.scalar.activation(
                            sg1, pg1, mybir.ActivationFunctionType.Sigmoid
                        )
                        if with_mask:
                            for j in range(2):
                                nc.gpsimd.tensor_scalar_mul(
                                    sg1[:, j, :], sg1[:, j, :],
                                    mask_sb[:, ts[j] : ts[j] + 1],
                                )
                        # out-gate -> go_raw: GPSIMD can't touch PSUM, so
                        # alternate ACT (sigmoid now) / DVE (raw copy,
                        # sigmoid later during phase 2) to balance engines
                        go_slice = go_raw.rearrange(
                            "k (r kc) d -> k kc r d", kc=KC
                        )[:, kc, r0 : r0 + 2, :]
                        if (kc * NPAIR + rp) % 2 == 0:
                            nc.scalar.activation(
                                go_slice, pg2,
                                mybir.ActivationFunctionType.Sigmoid,
                            )
                        else:
                            nc.vector.tensor_copy(go_slice, pg2)
                        # fused gating: LR[r, side, c] = vals[r, side*C + c]
                        #   * sig1[r, side*C + c]; side 0 = right(b), 1 = left(a)
                        out_ap = lr_loc[:, kc, :, :, r0 : r0 + 2].rearrange(
                            "k s c r -> k r s c"
                        )
                        nc.vector.tensor_tensor(
                            out_ap,
                            pv.rearrange("k r (s c) -> k r s c", s=2),
                            sg1.rearrange("k r (s c) -> k r s c", s=2),
                            mybir.AluOpType.mult,
                        )
                    # stage this kc chunk and kick its AllToAll
                    for s in range(2):
                        nc.sync.dma_start(
                            lra2a[kc][:, s].rearrange("g k cl i -> k g (cl i)"),
                            lr_loc[:, kc, s].rearrange("k (g cl) i -> k g (cl i)", g=W),
                        )
                    nc.gpsimd.collective_compute(
                        "AllToAll",
                        mybir.AluOpType.bypass,
                        replica_groups=[list(range(W))],
                        ins=[lra2a[kc][:]],
                        outs=[ga2a[kc][:]],
                    )

            p1pool.release()  # zs_sb, lr_loc freed (staged to DRAM)

            # ---------------- phase 2: channel-sharded einsum ----------------
            p2big = tc.alloc_tile_pool(name="p2big", bufs=1)
            gg = p2big.tile([P, NT, D], BF16)  # sigmoid(out-gate)
            zt = p2big.tile([C, NT * P], BF16)  # z_out, c on partitions
            zo_sb = [
                p2big.tile([P, CL, n], BF16, name=f"zo_sb{ib}") for ib in range(KC)
            ]

            with (
                tc.tile_pool(name="p2_l", bufs=3) as lpool,
                tc.tile_pool(name="p2_r", bufs=3) as rpool,
                tc.tile_pool(name="p2_ps", bufs=4, space="PSUM") as p2psum,
            ):
                # finish the out-gate: even pairs already hold sigmoid(go)
                # (Pool copies them), odd pairs get their sigmoid here on
                # the otherwise-idle ACT engine
                go_v = go_raw.rearrange("k (r kc) d -> k kc r d", kc=KC)
                gg_v = gg.rearrange("k (r kc) d -> k kc r d", kc=KC)
                for kc in range(KC):
                    for rp in range(NPAIR):
                        r0 = 2 * rp
                        src = go_v[:, kc, r0 : r0 + 2, :]
                        dst = gg_v[:, kc, r0 : r0 + 2, :]
                        if (kc * NPAIR + rp) % 2 == 0:
                            nc.gpsimd.tensor_copy(dst, src)
                        else:
                            nc.scalar.activation(
                                dst, src, mybir.ActivationFunctionType.Sigmoid
                            )
                for cl in range(CL):
                    lc = lpool.tile([P, KC, n], BF16, tag="lc")
                    rc = rpool.tile([P, KC, n], BF16, tag="rc")
                    for kc in range(KC):
                        nc.sync.dma_start(
                            lc[:, kc, :].rearrange("k (g i) -> k g i", g=W),
                            ga2a[kc][:, 1, :, cl, :].rearrange("g k i -> k g i"),
                        )
                        nc.sync.dma_start(
                            rc[:, kc, :].rearrange("k (g i) -> k g i", g=W),
                            ga2a[kc][:, 0, :, cl, :].rearrange("g k i -> k g i"),
                        )
                    for ib in range(KC):
                        ps = p2psum.tile([P, n], F32, tag="p2")
                        for kc in range(KC):
                            nc.tensor.matmul(
                                ps,
                                lhsT=lc[:, kc, ib * P : (ib + 1) * P],
                                rhs=rc[:, kc, :],
                                start=(kc == 0),
                                stop=(kc == KC - 1),
                            )
                        nc.vector.tensor_copy(zo_sb[ib][:, cl, :], ps)
                    # after each c-half completes: stage + backward AllToAll
                    if cl == CL // 2 - 1 or cl == CL - 1:
                        h = 0 if cl < CL // 2 else 1
                        c0 = h * CH
                        # rows g*SH..g*SH+SH-1 live in ib = row//P tiles
                        for g in range(W):
                            lo, hi = g * SH, (g + 1) * SH
                            while lo < hi:
                                ib = lo // P
                                seg = min(hi, (ib + 1) * P) - lo
                                nc.sync.dma_start(
                                    zoa2a[h][g]
                                    .rearrange("c i j -> i c j")[
                                        lo - g * SH : lo - g * SH + seg
                                    ],
                                    zo_sb[ib][lo - ib * P : lo - ib * P + seg,
                                              c0 : c0 + CH, :],
                                )
                                lo += seg
                        nc.gpsimd.collective_compute(
                            "AllToAll",
                            mybir.AluOpType.bypass,
                            replica_groups=[list(range(W))],
                            ins=[zoa2a[h][:]],
                            outs=[gza2a[h][:]],
                        )
                # gather z_out to [c, (i j)] with c on partitions
                for h in range(2):
                    for src in range(W):
                        nc.sync.dma_start(
                            zt[CL * src + CH * h : CL * src + CH * (h + 1), :],
                            gza2a[h][src].rearrange("c i j -> c (i j)"),
                        )

            # ---------------- phase 3: LN(z_out) @ Wo * gate ----------------
            stats = tc.alloc_tile_pool(name="p3stats", bufs=1, space="PSUM")
            S_ps = stats.tile([P, NT], F32)
            SQ_ps = stats.tile([P, NT], F32)
            with tc.tile_pool(name="p3_sq", bufs=3) as sqpool:
                for qg in range(NT // 4):
                    sq = sqpool.tile([P, 4, P], BF16, tag="sq")
                    zq = zt[:, 4 * P * qg : 4 * P * (qg + 1)].rearrange(
                        "c (t p) -> c t p", t=4
                    )
                    nc.vector.tensor_tensor(sq, zq, zq, mybir.AluOpType.mult)
                    for tl in range(4):
                        t3 = 4 * qg + tl
                        nc.tensor.matmul(
                            S_ps[:, t3 : t3 + 1],
                            lhsT=zt[:, t3 * P : (t3 + 1) * P],
                            rhs=ones_bf, start=True, stop=True,
                        )
                        nc.tensor.matmul(
                            SQ_ps[:, t3 : t3 + 1],
                            lhsT=sq[:, tl, :],
                            rhs=ones_bf, start=True, stop=True,
                        )

            p3big = tc.alloc_tile_pool(name="p3big", bufs=1)
            mean = p3big.tile([P, NT], F32)
            nc.vector.tensor_scalar_mul(mean, S_ps, 1.0 / C)
            msq = p3big.tile([P, NT], F32)
            nc.vector.tensor_scalar_mul(msq, SQ_ps, 1.0 / C)
            var = p3big.tile([P, NT], F32)
            nc.vector.tensor_tensor(var, mean, mean, mybir.AluOpType.mult)
            nc.vector.tensor_tensor(var, msq, var, mybir.AluOpType.subtract)
            rstd = p3big.tile([P, NT], F32)
            nc.scalar.activation(
                rstd, var, mybir.ActivationFunctionType.Sqrt, bias=eps_sb
            )
            nc.vector.reciprocal(rstd, rstd)
            stats.release()

            OCH = 16  # output tiles per DMA chunk
            with (
                tc.tile_pool(name="p3_ot", bufs=2) as otpool,
                tc.tile_pool(name="p3_po", bufs=4) as popool,
                tc.tile_pool(name="p3_ps", bufs=4, space="PSUM") as p3psum,
            ):
                for g in range(NT // OCH):
                    ot = otpool.tile([P, OCH, D], F32, tag="ot")
                    for tl in range(OCH):
                        t3 = g * OCH + tl
                        pr = p3psum.tile([P, D], F32, tag="pr")
                        nc.tensor.matmul(
                            pr, lhsT=zt[:, t3 * P : (t3 + 1) * P], rhs=wo_sb,
                            start=True, stop=True,
                        )
                        if with_bias:
                            po = popool.tile([P, D], F32, tag="pob")
                            nc.vector.tensor_scalar_mul(
                                po, pr, rstd[:, t3 : t3 + 1]
                            )
                            nc.vector.tensor_tensor(
                                po, po, bo_sb, mybir.AluOpType.add
                            )
                            nc.vector.tensor_tensor(
                                ot[:, tl, :], po, gg[:, t3, :],
                                mybir.AluOpType.mult,
                            )
                        elif t3 % 12 < 7:
                            # DVE: (pr * rstd) * gg straight from PSUM
                            nc.vector.scalar_tensor_tensor(
                                ot[:, tl, :], pr, rstd[:, t3 : t3 + 1],
                                gg[:, t3, :],
                                mybir.AluOpType.mult, mybir.AluOpType.mult,
                            )
                        else:
                            # ACT evacuates PSUM with the rstd scale; Pool
                            # (SBUF-only) applies the gate
                            po = popool.tile([P, D], BF16, tag="po")
                            nc.scalar.activation(
                                po, pr, mybir.ActivationFunctionType.Copy,
                                scale=rstd[:, t3 : t3 + 1],
                            )
                            nc.gpsimd.tensor_tensor(
                                ot[:, tl, :], po, gg[:, t3, :],
                                mybir.AluOpType.mult,
                            )
                    nc.sync.dma_start(out_sh[:, g * OCH : (g + 1) * OCH, :], ot)

            p3big.release()
            p2big.release()

    nc.compile()
    return nc


_CACHE = {}


def _get_nc(n, with_bias, with_mask):
    key = (n, with_bias, with_mask)
    if key not in _CACHE:
        _CACHE[key] = build_nc(n=n, with_bias=with_bias, with_mask=with_mask)
    return _CACHE[key]


def prepare_host(z, mask, norm_g, norm_b, norm_out_g, norm_out_b,
                 Wa, ba, Wb, bb, Wga, bga, Wgb, bgb, Wo, bo, Wgo, bgo, n=N_FULL):
    """Fold norm affines + centering into weights; pre-normalize z rows."""
    f = np.asarray
    z = f(z, dtype=np.float32)
    mask = f(mask, dtype=np.float32)
    g = f(norm_g, np.float32)
    b = f(norm_b, np.float32)
    go = f(norm_out_g, np.float32)
    bo_n = f(norm_out_b, np.float32)

    # LN(z) @ W_aff + bias = (z*rstd) @ Wcen + (b @ W + bias),
    # Wcen = (I - J/D)(g .* W)
    def fold(Wm, bias):
        Wm = f(Wm, np.float32)
        Wg = g[:, None] * Wm
        Wcen = Wg - np.mean(Wg, axis=0, keepdims=True)
        return Wcen, f(bias, np.float32) + b @ Wm

    Wa_, ba_ = fold(Wa, ba)
    Wga_, bga_ = fold(Wga, bga)
    Wb_, bb_ = fold(Wb, bb)
    Wgb_, bgb_ = fold(Wgb, bgb)
    Wgo_, bgo_ = fold(Wgo, bgo)
    Wo32 = f(Wo, np.float32)
    Wog = go[:, None] * Wo32
    Wo_ = Wog - np.mean(Wog, axis=0, keepdims=True)
    bo_ = f(bo, np.float32) + bo_n @ Wo32

    bf = ml_dtypes.bfloat16
    wva_h = np.concatenate([Wb_, Wa_], axis=1).astype(bf)
    wg3_h = np.concatenate([Wgb_, Wga_, Wgo_], axis=1).astype(bf)
    wo_h = Wo_.astype(bf)
    bva_h = np.concatenate([bb_, ba_])[None, :].astype(np.float32)
    bg3_h = np.concatenate([bgb_, bga_, bgo_])[None, :].astype(np.float32)

    with_bias = bool(np.any(bva_h) or np.any(bg3_h) or np.any(bo_))
    with_mask = not bool(np.all(mask == 1.0))

    # host-side LN stats: rstd per row of z, folded into z itself
    zf = z[0].reshape(n * n, D)
    m = zf.mean(axis=1, keepdims=True)
    v = ((zf - m) ** 2).mean(axis=1, keepdims=True)
    r = 1.0 / np.sqrt(v + EPS)
    zsf = (zf * r).astype(np.float32)

    SH = n // W
    NT = SH * n // P
    in_maps = []
    for mi in range(W):
        rows = zsf[SH * n * mi : SH * n * (mi + 1)]  # [SH*n, D]
        zs_h = np.ascontiguousarray(rows.T).astype(bf)  # [D, SH*n]
        im = {
            "zs": zs_h,
            "wva": wva_h,
            "wg3": wg3_h,
            "wo": wo_h,
        }
        if with_bias:
            im["bva"] = bva_h
            im["bg3"] = bg3_h
            im["bo"] = bo_[None, :].astype(np.float32)
        if with_mask:
            msk = mask[0].reshape(n * n)[SH * n * mi : SH * n * (mi + 1)]
            im["mask_sh"] = np.ascontiguousarray(
                msk.reshape(NT, P).T
            ).astype(np.float32)
        in_maps.append(im)
    return in_maps, with_bias, with_mask


def unshard(results, n=N_FULL):
    """results: list of per-core out_sh arrays [P, NT, D] -> [1, n, n, D]."""
    SH = n // W
    NT = SH * n // P
    parts = []
    for mi in range(W):
        o = results[mi].reshape(P, NT, D)
        parts.append(o.transpose(1, 0, 2).reshape(SH, n, D))
    return np.concatenate(parts, axis=0)[None]


def kernel(**inputs):
    n = inputs["z"].shape[1]
    in_maps, with_bias, with_mask = prepare_host(**inputs, n=n)
    nc = _get_nc(n, with_bias, with_mask)
    res = run_bass_kernel_spmd(nc, in_maps, list(range(W)))
    out = unshard([res.results[m]["out_sh"] for m in range(W)], n=n)
    return out.astype(np.float32)
